# revision 17
# baseline (speedup 1.0000x reference)
"""CRF loss kernel for nn_CRF_19086834663558 on 8 trn2 NeuronCores.

Strategy
--------
Phase A (all 8 cores, vocab-sharded): the three emission matrices are
uploaded host-transposed (rows = vocab columns of the original), sharded
by vocab range.  Each core row-gathers the columns its shard owns via
indirect DMA, computes its share of the gold-path emission potential with
an iota==y mask-reduce, and scatters the E-columns (bf16) into a slot
buffer which is AllReduced so every core holds E[:, x[t]] for all t.

Phase B (replicated on all cores): the alpha recursion
    a_t = (a_{t-1} @ Tm) * E[:, x[t]]
runs unnormalized in fp8:  W = fp8(16*Tm) stationary tiles on the PE,
state fp8 [128,4] column layout, per-step 16 accumulating N=1 matmuls +
one DVE multiply by ehat_t = 2^-11 * E[:, x[t]].  The 2^-7 per-step scale
(16 * 2^-11) cancels the ~2^7 natural growth; a sum-based renorm every 32
steps bounds the drift, accumulating ln(s/4096) into logacc.  At the end
  logz = ln(sum(a_final)) + logacc + (L-1)*7*ln(2).

The gold-path potential:  T/Cap parts are computed from host-side
bincount matrices (pure index arithmetic) contracted against T/Cap on
device; emission parts come from phase A's mask-reduce.

Host does only: layout transforms (transpose/reshape), index arithmetic
on x/y/upper (shard lists, bincounts), scalar constant addition at the
end, and the 8-way shard/unshard around one SPMD bass launch.
"""

import math
import os
import sys
import types

import numpy as np

# ---------------------------------------------------------------------------
# shims: missing antenv.axon_hooks module + walrus drain-wait-count workaround
# ---------------------------------------------------------------------------

_HOOK = [None]


def _install_axon_shim():
    if "antenv.axon_hooks" not in sys.modules:
        mod = types.ModuleType("antenv.axon_hooks")

        def set_axon_ntff_profile_hook(h):
            _HOOK[0] = h

        def get_axon_ntff_profile_hook():
            return _HOOK[0]

        mod.set_axon_ntff_profile_hook = set_axon_ntff_profile_hook
        mod.get_axon_ntff_profile_hook = get_axon_ntff_profile_hook
        sys.modules["antenv.axon_hooks"] = mod
        try:
            import antenv

            antenv.axon_hooks = mod
        except ImportError:
            pass
        try:
            from trn_agent_boot.trn_boot import _ntff_profile_via_ctypes

            hook = _ntff_profile_via_ctypes("/opt/axon/libaxon_pjrt.so")
            if hook is not None:
                mod.set_axon_ntff_profile_hook(hook)
        except Exception:
            pass


def _install_tile_patch():
    import concourse.tile as tile

    def _patched_drain_and_barrier(self, tick_clock, wait_clock):
        # the nix walrus rejects >2 sem waits on one CTRL (Drain) inst;
        # split the tile-exit drain into one drain per active proc.
        from bass_rust import ScopedClock, VectorClock

        ticks = list(tick_clock.global_clock)
        active = [i for i, t in enumerate(ticks) if t > 0]
        if not active:
            self.nc.sync.drain()
        for i in active:
            partial = [0] * len(ticks)
            partial[i] = ticks[i]
            d = self.nc.sync.drain()
            wait_clock.add_sem_waits(
                d.ins, ScopedClock({None: VectorClock(partial)})
            )
        self.nc.all_engine_barrier()
        popped = self.nc._tile_sem_poison_stack.pop()
        assert popped is self._sem_poison
        self.nc.clear_and_free_semaphores(list(self.sems.allocated().values()))
        self.nc.all_engine_barrier()

    tile.TileContext._drain_and_barrier = _patched_drain_and_barrier


_install_axon_shim()

_MAX_WAITS = 1


def _split_bir_waits(bir_bytes):
    """The nix walrus rejects instructions carrying more than ~2 sem waits.
    Hoist all but _MAX_WAITS waits of every instruction onto injected
    EventSemaphore instructions placed immediately before it (same engine
    stream, so ordering semantics are identical)."""
    import orjson

    bir = orjson.loads(bir_bytes)
    n_split = 0
    for fn in bir.get("functions", []):
        for bb in fn.get("blocks", []):
            insts = bb.get("instructions", [])
            out = []
            for inst in insts:
                si = inst.get("sync_info") or {}
                ow = si.get("on_wait") or []
                # raw-ISA instructions cannot carry any waits (encoded
                # length mismatch); others at most _MAX_WAITS
                maxw = 0 if inst.get("opcode") == "ISA" else _MAX_WAITS
                if len(ow) > maxw:
                    keep = ow[-maxw:] if maxw else []
                    hoist = ow[: len(ow) - maxw]
                    for i, w in enumerate(hoist):
                        out.append(
                            {
                                "engine": inst["engine"],
                                "ins": [],
                                "is_reset_sema": False,
                                "name": f"{inst['name']}_wsp{i}",
                                "opcode": "Drain",
                                "outs": [],
                                "sync_info": {"on_update": [], "on_wait": [w]},
                            }
                        )
                    si["on_wait"] = keep
                    inst["sync_info"] = si
                    n_split += len(hoist)
                out.append(inst)
            bb["instructions"] = out
    if n_split:
        print(f"bir waitsplit: hoisted {n_split} waits", file=sys.stderr)
    return orjson.dumps(bir)


def _install_waitsplit_patch():
    import concourse.bass2jax as bass2jax
    import concourse.bass_utils as bass_utils

    if getattr(bass2jax, "_waitsplit_patched", False):
        return
    orig = bass_utils.compile_bir_kernel

    def wrapped(bir_json, tmpdir, neff_name="file.neff"):
        return orig(_split_bir_waits(bir_json), tmpdir, neff_name)

    bass_utils.compile_bir_kernel = wrapped
    bass2jax.compile_bir_kernel = wrapped
    bass2jax._waitsplit_patched = True


# ---------------------------------------------------------------------------
# problem constants
# ---------------------------------------------------------------------------

M_TAGS = 512
V_VOCAB = 50000
B_IDX = 50000  # boundary column index into Eprev/Enext (and virtual T row)
L_SEQ = 4096
N_CORES = 8
VSH = 6250  # vocab values per shard (core 7 also owns B_IDX)
NSH = 6256  # padded rows per matrix shard (mult of 16)
F32MAX = np.float32(3.4e38)

LN2 = math.log(2.0)


def _cfg(L):
    """Derived sizes for sequence length L (L=4096 real, smaller for dev)."""
    nslot = ((L + 3) + 127) // 128 * 128  # t slots + 2 specials + trash
    if L >= 1024:
        U = 64
    else:
        U = 16
    niter = (L - 1) // U  # For_i iterations
    tail = (L - 1) - niter * U  # statically unrolled remainder steps
    # padded per-core entry count (3 streams + specials, multinomial max)
    npad = ((3 * L + 2) // N_CORES + 4 * int(math.sqrt(3 * L)) + 127) // 128 * 128
    return dict(L=L, NSLOT=nslot, U=U, NITER=niter, TAIL=tail, NPAD=npad)


# ---------------------------------------------------------------------------
# device program
# ---------------------------------------------------------------------------


def build_program(L=L_SEQ, debug=False):
    import concourse.bass as bass
    import concourse.mybir as mybir
    import concourse.tile as tile

    _install_tile_patch()

    cfg = _cfg(L)
    NSLOT, U, NITER, TAIL, NPAD = (
        cfg["NSLOT"],
        cfg["U"],
        cfg["NITER"],
        cfg["TAIL"],
        cfg["NPAD"],
    )
    NG = NPAD // 128  # gather calls
    NT = NSLOT // 128  # transpose tiles
    f32 = mybir.dt.float32
    bf16 = mybir.dt.bfloat16
    fp8 = mybir.dt.float8e4
    i32 = mybir.dt.int32

    nc = bass.Bass(num_devices=N_CORES)

    # --- dram tensors -----------------------------------------------------
    esh = nc.dram_tensor("esh", [3 * NSH, 512], f32, kind="ExternalInput")
    idxs = nc.dram_tensor("idxs", [NPAD, 1], i32, kind="ExternalInput")
    slots = nc.dram_tensor("slots", [NPAD, 1], i32, kind="ExternalInput")
    yvals = nc.dram_tensor("yvals", [NPAD, 1], f32, kind="ExternalInput")
    t_rows = nc.dram_tensor("t_rows", [4, 128, 512], f32, kind="ExternalInput")
    t_last = nc.dram_tensor("t_last", [1, 512], f32, kind="ExternalInput")
    t_last_col = nc.dram_tensor("t_last_col", [128, 4], f32, kind="ExternalInput")
    ct_rows = nc.dram_tensor("ct_rows", [4, 128, 512], f32, kind="ExternalInput")
    ct_last = nc.dram_tensor("ct_last", [1, 512], f32, kind="ExternalInput")
    cap_cols = nc.dram_tensor("cap_cols", [128, 4, 2], f32, kind="ExternalInput")
    ccap_cols = nc.dram_tensor("ccap_cols", [128, 4, 2], f32, kind="ExternalInput")
    cap_sel = nc.dram_tensor("cap_sel", [128, 2], f32, kind="ExternalInput")
    ones_col = nc.dram_tensor("ones_col", [128, 1], f32, kind="ExternalInput")
    bcast_row = nc.dram_tensor("bcast_row", [1, 128], f32, kind="ExternalInput")
    ident = nc.dram_tensor("ident", [128, 128], bf16, kind="ExternalInput")
    iota512 = nc.dram_tensor("iota512", [128, 512], f32, kind="ExternalInput")

    ex = nc.dram_tensor("ex", [NSLOT, 512], bf16)
    exo = nc.dram_tensor("exo", [NSLOT, 512], bf16, addr_space="Shared")
    out = nc.dram_tensor("out", [1, 4], f32, kind="ExternalOutput")
    if debug:
        dbg = nc.dram_tensor("dbg", [128, 32], f32, kind="ExternalOutput")

    with tile.TileContext(nc) as tc:
        with (
            tc.tile_pool(name="persist", bufs=1) as pp,
            tc.tile_pool(name="work", bufs=3) as wp,
            tc.tile_pool(name="psA", bufs=2, space="PSUM") as psA,
            tc.tile_pool(name="psB", bufs=2, space="PSUM") as psB,
            tc.tile_pool(name="psS", bufs=2, space="PSUM") as psS,
        ):
            # --- persistent sbuf ------------------------------------------
            ehat = pp.tile([128, 4, NSLOT], f32, tag="ehat")
            t_sb = pp.tile([128, 4, 512], f32, tag="t_sb")
            ct_sb = pp.tile([128, 4, 512], f32, tag="ct_sb")
            w_sb = pp.tile([128, 4, 4, 128], fp8, tag="w_sb")
            iota_sb = pp.tile([128, 512], f32, tag="iota_sb")
            idx_sb = pp.tile([128, NG], i32, tag="idx_sb")
            slot_sb = pp.tile([128, NG], i32, tag="slot_sb")
            y_sb = pp.tile([128, NG], f32, tag="y_sb")
            phiparts = pp.tile([128, NG], f32, tag="phiparts")
            ones_sb = pp.tile([128, 1], f32, tag="ones_sb")
            brow_sb = pp.tile([1, 128], f32, tag="brow_sb")
            ident_sb = pp.tile([128, 128], bf16, tag="ident_sb")
            tlast_sb = pp.tile([1, 512], f32, tag="tlast_sb")
            ctlast_sb = pp.tile([1, 512], f32, tag="ctlast_sb")
            tlastcol_sb = pp.tile([128, 4], f32, tag="tlastcol_sb")
            cap_sb = pp.tile([128, 4, 2], f32, tag="cap_sb")
            ccap_sb = pp.tile([128, 4, 2], f32, tag="ccap_sb")
            capsel_sb = pp.tile([128, 2], f32, tag="capsel_sb")
            aA = pp.tile([128, 4], fp8, tag="aA")
            aB = pp.tile([128, 4], fp8, tag="aB")
            colsum = pp.tile([128, 1], f32, tag="colsum")
            logacc = pp.tile([1, 1], f32, tag="logacc")
            ltmp = pp.tile([1, 1], f32, tag="ltmp")
            rcp = pp.tile([1, 1], f32, tag="rcp")
            rb_sb = pp.tile([128, 1], f32, tag="rb_sb")
            out_sb = pp.tile([1, 4], f32, tag="out_sb")
            if debug:
                dbg_sb = pp.tile([128, 32], f32, tag="dbg_sb")
            else:
                dbg_sb = None
            phig = pp.tile([128, 1], f32, tag="phig")
            phitc = pp.tile([128, 1], f32, tag="phitc")
            phicap = pp.tile([128, 1], f32, tag="phicap")
            philast = pp.tile([1, 1], f32, tag="philast")
            scrbig = pp.tile([128, 2048], f32, tag="scrbig")

            # --- small loads ----------------------------------------------
            nc.sync.dma_start(out=iota_sb[:], in_=iota512[:])
            nc.sync.dma_start(
                out=idx_sb[:],
                in_=idxs[:].rearrange("(k p) one -> p (k one)", p=128),
            )
            nc.sync.dma_start(
                out=slot_sb[:],
                in_=slots[:].rearrange("(k p) one -> p (k one)", p=128),
            )
            nc.sync.dma_start(
                out=y_sb[:],
                in_=yvals[:].rearrange("(k p) one -> p (k one)", p=128),
            )
            nc.sync.dma_start(out=ones_sb[:], in_=ones_col[:])
            nc.sync.dma_start(out=brow_sb[:], in_=bcast_row[:])
            nc.sync.dma_start(out=ident_sb[:], in_=ident[:])
            nc.sync.dma_start(out=tlast_sb[:], in_=t_last[:])
            nc.sync.dma_start(out=ctlast_sb[:], in_=ct_last[:])
            nc.sync.dma_start(out=tlastcol_sb[:], in_=t_last_col[:])
            nc.sync.dma_start(out=cap_sb[:], in_=cap_cols[:])
            nc.sync.dma_start(out=ccap_sb[:], in_=ccap_cols[:])
            nc.sync.dma_start(out=capsel_sb[:], in_=cap_sel[:])
            for i in range(4):
                nc.sync.dma_start(out=t_sb[:, i, :], in_=t_rows[i])
                nc.sync.dma_start(out=ct_sb[:, i, :], in_=ct_rows[i])

            # --- phase A: zero EX, then gather + mask-reduce + scatter ----
            zt = pp.tile([128, 512], bf16, tag="zt")
            nc.vector.memset(zt[:], 0.0)
            for k in range(NT):
                nc.sync.dma_start(
                    out=ex[128 * k : 128 * (k + 1), :], in_=zt[:]
                )
            for k in range(NG):
                g = wp.tile([128, 512], f32, tag="gath")
                nc.gpsimd.indirect_dma_start(
                    out=g[:],
                    out_offset=None,
                    in_=esh[:],
                    in_offset=bass.IndirectOffsetOnAxis(
                        ap=idx_sb[:, k : k + 1], axis=0
                    ),
                )
                gb = wp.tile([128, 512], bf16, tag="gathb")
                nc.vector.tensor_copy(gb[:], g[:])
                mask = wp.tile([128, 512], f32, tag="mask")
                nc.vector.tensor_tensor(
                    out=mask[:],
                    in0=y_sb[:, k : k + 1].to_broadcast([128, 512]),
                    in1=iota_sb[:],
                    op=mybir.AluOpType.is_equal,
                )
                scr = wp.tile([128, 512], f32, tag="scr")
                nc.vector.tensor_tensor(
                    out=scr[:], in0=mask[:], in1=g[:], op=mybir.AluOpType.mult
                )
                nc.vector.tensor_reduce(
                    out=phiparts[:, k : k + 1],
                    in_=scr[:],
                    axis=mybir.AxisListType.X,
                    op=mybir.AluOpType.add,
                )
                nc.gpsimd.indirect_dma_start(
                    out=ex[:],
                    out_offset=bass.IndirectOffsetOnAxis(
                        ap=slot_sb[:, k : k + 1], axis=0
                    ),
                    in_=gb[:],
                    in_offset=None,
                )

            nc.gpsimd.collective_compute(
                "AllReduce",
                mybir.AluOpType.add,
                replica_groups=[list(range(N_CORES))],
                ins=[ex[:]],
                outs=[exo[:]],
            )

            # --- phase A2: transpose E-columns into tag-major ehat --------
            for k in range(NT):
                exsb = wp.tile([128, 512], bf16, tag="exsb")
                nc.sync.dma_start(
                    out=exsb[:], in_=exo[128 * k : 128 * (k + 1), :]
                )
                for j in range(4):
                    ps = psA.tile([128, 128], bf16, tag="trps")
                    nc.tensor.transpose(
                        ps[:], exsb[:, 128 * j : 128 * (j + 1)], ident_sb[:]
                    )
                    nc.vector.tensor_scalar_mul(
                        ehat[:, j, 128 * k : 128 * (k + 1)], ps[:], 2.0 ** -11
                    )

            # --- phase A3: weights, phi T/Cap parts, alpha0 ---------------
            for i in range(4):
                nc.vector.tensor_scalar_mul(w_sb[:, i], t_sb[:, i], 16.0)

            nc.vector.tensor_tensor(
                out=scrbig[:],
                in0=t_sb[:].rearrange("p a b -> p (a b)"),
                in1=ct_sb[:].rearrange("p a b -> p (a b)"),
                op=mybir.AluOpType.mult,
            )
            nc.vector.tensor_reduce(
                out=phitc[:],
                in_=scrbig[:],
                axis=mybir.AxisListType.X,
                op=mybir.AluOpType.add,
            )
            scr1 = wp.tile([1, 512], f32, tag="scr1")
            nc.vector.tensor_tensor(
                out=scr1[:], in0=tlast_sb[:], in1=ctlast_sb[:], op=mybir.AluOpType.mult
            )
            nc.vector.tensor_reduce(
                out=philast[:],
                in_=scr1[:],
                axis=mybir.AxisListType.X,
                op=mybir.AluOpType.add,
            )
            scr8 = wp.tile([128, 8], f32, tag="scr8")
            nc.vector.tensor_tensor(
                out=scr8[:],
                in0=cap_sb[:].rearrange("p a b -> p (a b)"),
                in1=ccap_sb[:].rearrange("p a b -> p (a b)"),
                op=mybir.AluOpType.mult,
            )
            nc.vector.tensor_reduce(
                out=phicap[:],
                in_=scr8[:],
                axis=mybir.AxisListType.X,
                op=mybir.AluOpType.add,
            )

            # alpha0 = exp(T[M]col + E[:,x0] + Enext[:,x1] + Eprev[:,B] + Cap[:,u0])
            e3 = wp.tile([128, 4], f32, tag="e3")
            nc.vector.tensor_add(e3[:], ehat[:, :, 0], ehat[:, :, L])
            nc.vector.tensor_add(e3[:], e3[:], ehat[:, :, L + 1])
            nc.vector.tensor_scalar_mul(e3[:], e3[:], 2048.0)
            nc.vector.tensor_add(e3[:], e3[:], tlastcol_sb[:])
            c0 = wp.tile([128, 4], f32, tag="c0")
            nc.vector.tensor_scalar(
                out=c0[:],
                in0=cap_sb[:, :, 0],
                scalar1=capsel_sb[:, 0:1],
                scalar2=None,
                op0=mybir.AluOpType.mult,
            )
            nc.vector.tensor_add(e3[:], e3[:], c0[:])
            nc.vector.tensor_scalar(
                out=c0[:],
                in0=cap_sb[:, :, 1],
                scalar1=capsel_sb[:, 1:2],
                scalar2=None,
                op0=mybir.AluOpType.mult,
            )
            nc.vector.tensor_add(e3[:], e3[:], c0[:])
            a0f = wp.tile([128, 4], f32, tag="a0f")
            nc.scalar.activation(
                a0f[:], e3[:], mybir.ActivationFunctionType.Exp
            )
            nc.vector.tensor_copy(aA[:], a0f[:])
            nc.vector.memset(logacc[:], 0.0)
            if debug:
                nc.vector.tensor_copy(dbg_sb[:, 0:4], a0f[:])
                nc.vector.tensor_copy(dbg_sb[:, 4:8], aA[:])
                nc.vector.tensor_copy(dbg_sb[:, 8:12], ehat[:, :, 1])
                nc.vector.tensor_copy(dbg_sb[:, 24:28], ehat[:, :, 2])

            # --- phase B: the recursion -----------------------------------
            def renorm(a_cur):
                # s = sum(a) (colsum already holds per-partition sums)
                sps = psS.tile([128, 1], f32, tag="scal")
                nc.tensor.matmul(
                    sps[0:1, 0:1], lhsT=ones_sb[:], rhs=colsum[:], start=True, stop=True
                )
                nc.scalar.activation(
                    ltmp[:],
                    sps[0:1, 0:1],
                    mybir.ActivationFunctionType.Ln,
                    scale=2.0 ** -12,
                )
                nc.vector.tensor_add(logacc[:], logacc[:], ltmp[:])
                nc.vector.reciprocal(rcp[:], sps[0:1, 0:1])
                bps = psS.tile([128, 1], f32, tag="scal")
                nc.tensor.matmul(
                    bps[:], lhsT=brow_sb[:], rhs=rcp[:], start=True, stop=True
                )
                nc.vector.tensor_copy(rb_sb[:], bps[:])
                nc.vector.tensor_scalar(
                    out=a_cur[:],
                    in0=a_cur[:],
                    scalar1=rb_sb[:, 0:1],
                    scalar2=None,
                    op0=mybir.AluOpType.mult,
                )

            def step(eh, a_cur, a_nxt, with_sum):
                ps4 = psB.tile([128, 4], f32, tag="ps4")
                step.last_ps4 = ps4
                for i in range(4):
                    for j in range(4):
                        nc.tensor.matmul(
                            ps4[:, j : j + 1],
                            lhsT=w_sb[:, i, j, :],
                            rhs=a_cur[:, i : i + 1],
                            start=(i == 0 and j == 0),
                            stop=(i == 3 and j == 3),
                            skip_group_check=True,
                        )
                nc.vector.tensor_tensor(
                    out=a_nxt[:],
                    in0=ps4[:],
                    in1=eh,
                    op=mybir.AluOpType.mult,
                )
                if with_sum:
                    nc.vector.tensor_reduce(
                        out=colsum[:],
                        in_=a_nxt[:],
                        axis=mybir.AxisListType.X,
                        op=mybir.AluOpType.add,
                    )

            REN = 32  # renorm period
            for t in range(1, L):
                s = t - 1
                a_cur, a_nxt = (aA, aB) if s % 2 == 0 else (aB, aA)
                last = t == L - 1
                rn = (t % REN) == 0 and not last
                step(ehat[:, :, t], a_cur, a_nxt, rn or last)
                if debug and t == 1:
                    nc.vector.tensor_copy(dbg_sb[:, 12:16], step.last_ps4[:])
                    nc.vector.tensor_copy(dbg_sb[:, 16:20], a_nxt[:])
                if rn:
                    renorm(a_nxt)
                    if debug and t == REN:
                        nc.vector.tensor_copy(dbg_sb[:, 20:21], colsum[:])
                        nc.vector.tensor_copy(dbg_sb[:, 21:22], rb_sb[:])
                        nc.vector.tensor_copy(dbg_sb[0:1, 22:23], ltmp[:])
                        nc.vector.tensor_copy(dbg_sb[:, 28:32], a_nxt[:])

            # --- finalize -------------------------------------------------
            sfin = psS.tile([128, 1], f32, tag="scal")
            nc.tensor.matmul(
                sfin[0:1, 0:1], lhsT=ones_sb[:], rhs=colsum[:], start=True, stop=True
            )
            nc.scalar.activation(
                out_sb[0:1, 0:1], sfin[0:1, 0:1], mybir.ActivationFunctionType.Ln
            )
            nc.vector.tensor_add(out_sb[0:1, 0:1], out_sb[0:1, 0:1], logacc[:])

            # phi partials
            nc.vector.tensor_reduce(
                out=phig[:],
                in_=phiparts[:],
                axis=mybir.AxisListType.X,
                op=mybir.AluOpType.add,
            )
            gps = psS.tile([128, 1], f32, tag="scal")
            nc.tensor.matmul(
                gps[0:1, 0:1], lhsT=ones_sb[:], rhs=phig[:], start=True, stop=True
            )
            nc.vector.tensor_copy(out_sb[0:1, 1:2], gps[0:1, 0:1])

            nc.vector.tensor_add(phitc[:], phitc[:], phicap[:])
            tps = psS.tile([128, 1], f32, tag="scal")
            nc.tensor.matmul(
                tps[0:1, 0:1], lhsT=ones_sb[:], rhs=phitc[:], start=True, stop=True
            )
            nc.vector.tensor_add(philast[:], philast[:], tps[0:1, 0:1])
            nc.vector.tensor_copy(out_sb[0:1, 2:3], philast[:])
            nc.vector.memset(out_sb[0:1, 3:4], 0.0)

            if debug:
                nc.vector.tensor_copy(dbg_sb[:, 23:24], colsum[:])
                nc.sync.dma_start(out=dbg[:], in_=dbg_sb[:])
            nc.sync.dma_start(out=out[:], in_=out_sb[:])

    return nc, cfg


# ---------------------------------------------------------------------------
# host-side input prep (layout + index arithmetic only)
# ---------------------------------------------------------------------------


def prep_inputs(T, E, Eprev, Enext, Cap, x, y, upper, L=L_SEQ):
    cfg = _cfg(L)
    NSLOT, NPAD = cfg["NSLOT"], cfg["NPAD"]
    TRASH = L + 2

    T = np.asarray(T, np.float32)
    E = np.asarray(E, np.float32)
    Eprev = np.asarray(Eprev, np.float32)
    Enext = np.asarray(Enext, np.float32)
    Cap = np.asarray(Cap, np.float32)
    x = np.asarray(x, np.int64)[:L]
    y = np.asarray(y, np.int64)[:L]
    upper = np.asarray(upper, np.int64)[:L]

    # transposed emission matrices, vocab-sharded (pure layout transform)
    ET = np.ascontiguousarray(E.T)  # [V, 512]
    EPT = np.ascontiguousarray(Eprev.T)  # [V+1, 512]
    ENT = np.ascontiguousarray(Enext.T)  # [V+1, 512]

    esh_all = []
    for c in range(N_CORES):
        lo = c * VSH
        hi = min((c + 1) * VSH + (1 if c == N_CORES - 1 else 0), V_VOCAB + 1)
        buf = np.zeros((3 * NSH, 512), np.float32)
        n = min(hi, V_VOCAB) - lo
        buf[0:n] = ET[lo : lo + n]
        buf[NSH : NSH + (hi - lo)] = EPT[lo:hi]
        buf[2 * NSH : 2 * NSH + (hi - lo)] = ENT[lo:hi]
        esh_all.append(buf)

    # entry streams (index arithmetic)
    xp = np.concatenate([[B_IDX], x[:-1]])
    xn = np.concatenate([x[1:], [B_IDX]])
    ent_core = [[] for _ in range(N_CORES)]  # (row, slot, y)

    def shard_of(v):
        return min(int(v) // VSH, N_CORES - 1)

    def local(v):
        return int(v) - shard_of(v) * VSH

    for t in range(L):
        v = x[t]
        ent_core[shard_of(v)].append((local(v), t, float(y[t])))
        v = xp[t]
        ent_core[shard_of(v)].append((NSH + local(v), TRASH, float(y[t])))
        v = xn[t]
        ent_core[shard_of(v)].append((2 * NSH + local(v), TRASH, float(y[t])))
    # specials: Enext[:, x[1]] -> slot L ; Eprev[:, B] -> slot L+1
    ent_core[shard_of(x[1])].append((2 * NSH + local(x[1]), L, -1.0))
    ent_core[shard_of(B_IDX)].append((NSH + local(B_IDX), L + 1, -1.0))

    idxs_all, slots_all, yv_all = [], [], []
    for c in range(N_CORES):
        ents = ent_core[c]
        assert len(ents) <= NPAD, f"core {c}: {len(ents)} > NPAD={NPAD}"
        ents = ents + [(0, TRASH, -1.0)] * (NPAD - len(ents))
        arr = np.array([e[0] for e in ents], np.int32).reshape(NPAD, 1)
        sl = np.array([e[1] for e in ents], np.int32).reshape(NPAD, 1)
        yv = np.array([e[2] for e in ents], np.float32).reshape(NPAD, 1)
        idxs_all.append(arr)
        slots_all.append(sl)
        yv_all.append(yv)

    # bincount matrices for the T / Cap parts of the path potential
    yprev = np.concatenate([[M_TAGS], y[:-1]])
    CT = np.zeros((M_TAGS + 1, M_TAGS), np.float32)
    np.add.at(CT, (yprev, y), 1.0)
    CCap = np.zeros((M_TAGS, 2), np.float32)
    np.add.at(CCap, (y, upper), 1.0)

    t_rows = T[:512].reshape(4, 128, 512)
    t_last = T[512:513].copy()
    t_last_col = np.ascontiguousarray(T[512].reshape(4, 128).T)
    ct_rows = CT[:512].reshape(4, 128, 512)
    ct_last = CT[512:513].copy()
    cap_cols = np.ascontiguousarray(Cap.reshape(4, 128, 2).transpose(1, 0, 2))
    ccap_cols = np.ascontiguousarray(CCap.reshape(4, 128, 2).transpose(1, 0, 2))
    cap_sel = np.zeros((128, 2), np.float32)
    cap_sel[:, int(upper[0])] = 1.0

    iota = np.broadcast_to(
        np.arange(512, dtype=np.float32)[None, :], (128, 512)
    ).copy()
    import ml_dtypes

    common = dict(
        t_rows=t_rows.astype(np.float32),
        t_last=t_last,
        t_last_col=t_last_col,
        ct_rows=ct_rows.astype(np.float32),
        ct_last=ct_last,
        cap_cols=cap_cols,
        ccap_cols=ccap_cols,
        cap_sel=cap_sel,
        ones_col=np.ones((128, 1), np.float32),
        bcast_row=np.full((1, 128), 4096.0, np.float32),
        ident=np.eye(128, dtype=ml_dtypes.bfloat16),
        iota512=iota,
    )

    in_maps = []
    for c in range(N_CORES):
        m = dict(common)
        m["esh"] = esh_all[c]
        m["idxs"] = idxs_all[c]
        m["slots"] = slots_all[c]
        m["yvals"] = yv_all[c]
        in_maps.append(m)
    return in_maps, cfg


def assemble(results, L=L_SEQ):
    """Combine per-core outputs into the final float32 scalar."""
    outs = [np.asarray(r["out"], np.float64).reshape(4) for r in results]
    logz = outs[0][0] + (L - 1) * 7.0 * LN2
    phi = sum(o[1] for o in outs) + outs[0][2]
    return np.float32(logz - phi)


# ---------------------------------------------------------------------------
# kernel entry point
# ---------------------------------------------------------------------------

_CACHE = {}


def _get_program(L=L_SEQ, debug=False):
    key = (L, debug)
    if key not in _CACHE:
        _CACHE[key] = build_program(L, debug=debug)
    return _CACHE[key]


def run_on_device(inputs, L=L_SEQ, trace=False, debug=False):
    from concourse.bass_utils import run_bass_kernel_spmd

    _install_waitsplit_patch()
    nc, cfg = _get_program(L, debug)
    in_maps, _ = prep_inputs(**inputs, L=L)
    res = run_bass_kernel_spmd(
        nc, in_maps, core_ids=list(range(N_CORES)), trace=trace
    )
    return assemble(res.results, L=L), res


def kernel(T, E, Eprev, Enext, Cap, x, y, upper):
    val, _ = run_on_device(
        dict(T=T, E=E, Eprev=Eprev, Enext=Enext, Cap=Cap, x=x, y=y, upper=upper),
        L=L_SEQ,
        trace=False,
    )
    return val


# ---------------------------------------------------------------------------
# dev helpers
# ---------------------------------------------------------------------------


def _numpy_reference(T, E, Eprev, Enext, Cap, x, y, upper, L=None):
    T = np.asarray(T, np.float64)
    E = np.asarray(E, np.float64)
    Eprev = np.asarray(Eprev, np.float64)
    Enext = np.asarray(Enext, np.float64)
    Cap = np.asarray(Cap, np.float64)
    x = np.asarray(x, np.int64)
    y = np.asarray(y, np.int64)
    upper = np.asarray(upper, np.int64)
    if L is not None:
        x, y, upper = x[:L], y[:L], upper[:L]
    M = T.shape[1]
    B = M  # NB: reference uses B = M_TAGS... careful: B is vocab boundary
    L = x.shape[0]
    phi0 = T[M_TAGS] + Eprev[:, B_IDX] + Enext[:, x[1]] + Cap[:, upper[0]] + E[:, x[0]]
    alpha0 = np.exp(phi0)
    s0 = alpha0.sum()
    beta = alpha0 / s0
    logz = np.log(s0)
    Tm = np.ascontiguousarray(T[:M_TAGS])
    for t in range(1, L):
        alpha = (beta @ Tm) * E[:, x[t]]
        s = alpha.sum()
        beta = alpha / s
        logz += np.log(s)
    yp = np.concatenate([[M_TAGS], y[:-1]])
    xpr = np.concatenate([[B_IDX], x[:-1]])
    xnx = np.concatenate([x[1:], [B_IDX]])
    phi = (
        T[yp, y].sum()
        + Eprev[y, xpr].sum()
        + Enext[y, xnx].sum()
        + Cap[y, upper].sum()
        + E[y, x].sum()
    )
    return np.float64(logz - phi)


if __name__ == "__main__":
    import argparse

    ap = argparse.ArgumentParser()
    ap.add_argument("--L", type=int, default=64)
    ap.add_argument("--build-only", action="store_true")
    ap.add_argument("--trace", action="store_true")
    args = ap.parse_args()

    rng = np.random.default_rng(0)
    ins = dict(
        T=rng.random((513, 512), np.float32),
        E=rng.random((512, 50000), np.float32),
        Eprev=(rng.standard_normal((512, 50001)) * 0.1).astype(np.float32),
        Enext=(rng.standard_normal((512, 50001)) * 0.1).astype(np.float32),
        Cap=(rng.standard_normal((512, 2)) * 0.1).astype(np.float32),
        x=rng.integers(0, 50000, 4096).astype(np.int32),
        y=rng.integers(0, 512, 4096).astype(np.int32),
        upper=rng.integers(0, 2, 4096).astype(np.int32),
    )
    if args.build_only:
        import time

        t0 = time.time()
        nc, cfg = build_program(args.L)
        print("built in", time.time() - t0, "cfg", cfg)
        sys.exit(0)

    expected = _numpy_reference(**ins, L=args.L)
    val, res = run_on_device(ins, L=args.L, trace=args.trace)
    rel = abs(float(val) - float(expected)) / max(1e-12, abs(float(expected)))
    print("expected", expected, "actual", val, "rel", rel)
    if res.exec_time_ns:
        print("HW exec time:", res.exec_time_ns, "ns")
